# revision 17
# baseline (speedup 1.0000x reference)
"""Trainium2 Bass kernel for nn_ConvLogicLayer.

Computes y[n,c,oy,ox,p] = k0 + ka*A + kb*B + kab*A*B where A/B are
shifted-window gathers of input channels (per the packed `selection`),
and k* are per-(c,p) coefficients derived from softmax(weights) @ OP_COEFFS.

Strategy:
  - Shard C_out (512) across 8 cores -> 64 output channels per core.
  - Each core gets a specialized program: the gather indices and the
    coefficients are baked into the instruction stream (static access
    patterns + immediate scalars), so the kernel is pure streaming
    elementwise work with zero gather traffic.
  - SBUF layout: partition q = n*4 + oyblk (32 images x 4 row-blocks),
    free dim = all 64 input channels x 10 halo rows x 34 padded cols.
    A shifted 8x32 window for any (ch,ry,rx) is then a single static
    3D access pattern on one SBUF tile.
  - Per (c,p) pair: u = kab*B + ka (ScalarE), v = kb*B + k0 (VectorE or
    ScalarE, load-balanced), w = u*A (VectorE), y = w + v written
    p-interleaved (VectorE).  Output DMAd per channel in 512KB chunks
    that are 4KB-contiguous in HBM.
"""

import os
import sys
import threading

import numpy as np

for _p in ("/opt/trn_rl_repo",):
    if _p not in sys.path and os.path.isdir(_p):
        sys.path.insert(0, _p)

import concourse.bass as bass
import concourse.bacc as bacc
import concourse.mybir as mybir
from concourse.tile import TileContext
from concourse import bass_utils

# Problem constants (hardcoded per spec)
N, C_IN, H, W = 32, 64, 32, 32
C_OUT, KPAIRS = 512, 4
N_CORES = 8
CPC = C_OUT // N_CORES  # channels per core

P = 128          # partitions = (n=32) x (oyblk=4)
OYB = 4          # oy blocks per image
OYS = 8          # oy rows per block
HALO = 10        # rows stored per block (8 + 2 halo)
W34 = 34         # padded width
CHSZ = HALO * W34           # 340 elems per (q, channel)
XFREE = C_IN * CHSZ         # 21760 elems per partition
OUT_CSTRIDE = H * W * KPAIRS          # 4096
OUT_NSTRIDE = CPC * OUT_CSTRIDE       # 262144

OP_COEFFS = np.array([
    [0.0, 0.0, 0.0, 0.0], [0.0, 0.0, 0.0, 1.0], [0.0, 1.0, 0.0, -1.0],
    [0.0, 1.0, 0.0, 0.0], [0.0, 0.0, 1.0, -1.0], [0.0, 0.0, 1.0, 0.0],
    [0.0, 1.0, 1.0, -2.0], [0.0, 1.0, 1.0, -1.0], [1.0, -1.0, -1.0, 1.0],
    [1.0, -1.0, -1.0, 2.0], [1.0, 0.0, -1.0, 0.0], [1.0, 0.0, -1.0, 1.0],
    [1.0, -1.0, 0.0, 0.0], [1.0, -1.0, 0.0, 1.0], [1.0, 0.0, 0.0, -1.0],
    [1.0, 0.0, 0.0, 0.0],
], dtype=np.float64)

MULT = mybir.AluOpType.mult
ADD = mybir.AluOpType.add
COPY = mybir.ActivationFunctionType.Copy

# Cost-model ns for load balancing (f32, [128, 256] tiles)
DVE_TT = 327.0   # tensor_tensor, 1x
DVE_TS = 194.0   # tensor_scalar, 2x_2P
ACT_TS = 507.0   # activation, 1x + 352cyc overhead
GPS_TS = 600.0   # gpsimd tensor_scalar (sw impl efficiency ~0.6)
GPS_TT = 600.0   # gpsimd tensor_tensor (sw impl efficiency ~0.42)

# Tuning knobs (A/B'd via TimelineSim; best found = 180.7us/core)
CFG = {
    "use_gps": True,     # offload v/y ops to GPSIMD
    "tp_bufs": 6,
    "yc_bufs": 3,
    "u_act_only": True,  # u always on ScalarE
    "w_dve_only": True,  # w always on VectorE
}

last_results = [None] * N_CORES  # BassKernelResults per core (for profiling)


def build_core_program(core, ch, ry, rx, coef):
    """One specialized Bass program for `core` (channels core*CPC..+CPC)."""
    nc = bacc.Bacc("TRN2", target_bir_lowering=False)
    xh_d = nc.dram_tensor("xh", [P, XFREE], mybir.dt.float32, kind="ExternalInput")
    out_d = nc.dram_tensor(
        "out", [N, CPC, H, W, KPAIRS], mybir.dt.float32, kind="ExternalOutput"
    )

    use_gps = CFG["use_gps"]
    eng_ns = {"dve": 0.0, "act": 0.0, "gps": 0.0}

    with TileContext(nc) as tc:
        with (
            tc.tile_pool(name="xp", bufs=1) as xpool,
            tc.tile_pool(name="tp", bufs=CFG["tp_bufs"]) as tpool,
            tc.tile_pool(name="yp", bufs=CFG["yc_bufs"]) as ypool,
        ):
            xh = xpool.tile([P, XFREE], mybir.dt.float32)
            nc.sync.dma_start(xh[:], xh_d[:])
            # Compute instructions on TRN2 carry at most ONE fused sync wait
            # ("Too many sync wait commands" in walrus otherwise).  Have each
            # compute engine absorb the load's DMA semaphore with one tiny op
            # so later ops never need the DMA wait *plus* a cross-engine wait.
            absd = xpool.tile([P, 4], mybir.dt.float32, tag="abs")
            nc.vector.tensor_scalar(absd[:, 0:1], xh[:, 0:1], 1.0, None, MULT)
            nc.scalar.activation(absd[:, 1:2], xh[:, 0:1], COPY, bias=0.0, scale=1.0)
            if use_gps:
                nc.gpsimd.tensor_scalar(absd[:, 2:3], xh[:, 0:1], 1.0, None, MULT)
            base = xh[:]
            pitch = base.ap[0][0]
            tens = base.tensor
            base_off = base.offset

            for cl in range(CPC):
                c = core * CPC + cl
                yc = ypool.tile([P, OYS * W * KPAIRS], mybir.dt.float32, tag="yc")
                ybase = yc[:]
                ypitch = ybase.ap[0][0]
                for p4 in range(KPAIRS):
                    ka_, kb_ = 2 * p4, 2 * p4 + 1
                    offA = base_off + int(ch[c, ka_]) * CHSZ + int(ry[c, ka_]) * W34 + int(rx[c, ka_])
                    offB = base_off + int(ch[c, kb_]) * CHSZ + int(ry[c, kb_]) * W34 + int(rx[c, kb_])
                    A_ap = bass.AP(tens, offA, [[pitch, P], [W34, OYS], [1, W]])
                    B_ap = bass.AP(tens, offB, [[pitch, P], [W34, OYS], [1, W]])

                    k0 = float(coef[c, p4, 0])
                    ka = float(coef[c, p4, 1])
                    kb = float(coef[c, p4, 2])
                    kab = float(coef[c, p4, 3])

                    u = tpool.tile([P, OYS * W], mybir.dt.float32, tag="u")
                    v = tpool.tile([P, OYS * W], mybir.dt.float32, tag="v")
                    w = tpool.tile([P, OYS * W], mybir.dt.float32, tag="w")
                    u3 = u[:].rearrange("p (a b) -> p a b", b=W)
                    v3 = v[:].rearrange("p (a b) -> p a b", b=W)
                    w3 = w[:].rearrange("p (a b) -> p a b", b=W)

                    def pick(cands):
                        eng, cost = min(cands, key=lambda c: eng_ns[c[0]] + c[1])
                        eng_ns[eng] += cost
                        return eng

                    # u = kab*B + ka
                    if CFG.get("u_act_only"):
                        ucands = [("act", ACT_TS)]
                    else:
                        ucands = [("act", ACT_TS), ("dve", DVE_TS)]
                        if use_gps:
                            ucands.append(("gps", GPS_TS))
                    ueng = pick(ucands)
                    if ueng == "act":
                        nc.scalar.activation(u3, B_ap, COPY, bias=ka, scale=kab)
                    elif ueng == "gps":
                        nc.gpsimd.tensor_scalar(u3, B_ap, kab, ka, MULT, ADD)
                    else:
                        nc.vector.tensor_scalar(u3, B_ap, kab, ka, MULT, ADD)
                    # v = kb*B + k0
                    vcands = [("dve", DVE_TS), ("act", ACT_TS)]
                    if use_gps:
                        vcands.append(("gps", GPS_TS))
                    veng = pick(vcands)
                    if veng == "act":
                        nc.scalar.activation(v3, B_ap, COPY, bias=k0, scale=kb)
                    elif veng == "gps":
                        nc.gpsimd.tensor_scalar(v3, B_ap, kb, k0, MULT, ADD)
                    else:
                        nc.vector.tensor_scalar(v3, B_ap, kb, k0, MULT, ADD)
                    # w = u * A
                    wcands = [("dve", DVE_TT)]
                    if use_gps and not CFG.get("w_dve_only"):
                        wcands.append(("gps", GPS_TT))
                    weng = pick(wcands)
                    if weng == "gps":
                        nc.gpsimd.tensor_tensor(w3, u3, A_ap, MULT)
                    else:
                        nc.vector.tensor_tensor(w3, u3, A_ap, MULT)
                    # y = w + v, written p-interleaved into yc
                    yap = bass.AP(
                        ybase.tensor, ybase.offset + p4,
                        [[ypitch, P], [W * KPAIRS, OYS], [KPAIRS, W]],
                    )
                    ycands = [("dve", DVE_TT)]
                    if use_gps:
                        ycands.append(("gps", GPS_TT))
                    yeng = pick(ycands)
                    if yeng == "gps":
                        nc.gpsimd.tensor_tensor(yap, w3, v3, ADD)
                    else:
                        nc.vector.tensor_tensor(yap, w3, v3, ADD)

                # DMA this channel out: HBM [n, oyblk, (oy',ox,p)=1024]
                oap = bass.AP(
                    out_d, cl * OUT_CSTRIDE,
                    [[OUT_NSTRIDE, N], [OYS * W * KPAIRS, OYB], [1, OYS * W * KPAIRS]],
                )
                nc.sync.dma_start(oap, yc[:])
    nc.finalize()  # Bacc: splits >1-wait syncs into event semaphores
    return nc


def _prep_inputs(x, weights, selection):
    x = np.ascontiguousarray(np.asarray(x, dtype=np.float32))
    weights = np.asarray(weights, dtype=np.float32)
    selection = np.asarray(selection, dtype=np.int32)

    # coefficients: softmax over 16 logic ops folded into {1,a,b,ab} basis
    w64 = weights.astype(np.float64)
    e = np.exp(w64 - w64.max(axis=-1, keepdims=True))
    prob = e / e.sum(axis=-1, keepdims=True)
    coef = (prob @ OP_COEFFS).astype(np.float32)  # [C_OUT, 4, 4]

    ch = ((selection >> 16) & 0xFFFF).astype(np.int64)
    ry = ((selection >> 8) & 0xFF).astype(np.int64)
    rx = (selection & 0xFF).astype(np.int64)

    # halo layout: xh[q=(n,oyblk), ch, r, w] = xpad[n, ch, oyblk*8+r, w]
    xpad = np.zeros((N, C_IN, H + 2, W + 2), dtype=np.float32)
    xpad[:, :, 1 : H + 1, 1 : W + 1] = x
    xh = np.empty((N, OYB, C_IN, HALO, W34), dtype=np.float32)
    for b in range(OYB):
        xh[:, b] = xpad[:, :, b * OYS : b * OYS + HALO, :]
    xh = np.ascontiguousarray(xh.reshape(P, XFREE))
    return xh, ch, ry, rx, coef


def kernel(x, weights, selection):
    assert x.shape == (N, C_IN, H, W), x.shape
    assert weights.shape == (C_OUT, 4, 16), weights.shape
    assert selection.shape == (C_OUT, 8), selection.shape

    xh, ch, ry, rx, coef = _prep_inputs(x, weights, selection)

    progs = [build_core_program(k, ch, ry, rx, coef) for k in range(N_CORES)]

    import jax

    devices = jax.devices()
    assert len(devices) >= N_CORES, devices

    outs = [None] * N_CORES
    errs = [None] * N_CORES
    # NTFF tracing needs axon hooks that aren't present in this container —
    # make sure run_bass_kernel_spmd never tries (BASS_TRACE in env would).
    os.environ["BASS_NEVER_TRACE"] = "1"

    def run_one(k):
        try:
            with jax.default_device(devices[k]):
                res = bass_utils.run_bass_kernel_spmd(
                    progs[k], [{"xh": xh}], core_ids=[k]
                )
            last_results[k] = res
            outs[k] = res.results[0]["out"]
        except Exception as e:  # noqa: BLE001
            errs[k] = e

    threads = [threading.Thread(target=run_one, args=(k,)) for k in range(N_CORES)]
    for t in threads:
        t.start()
    for t in threads:
        t.join()
    for k, e in enumerate(errs):
        if e is not None:
            raise RuntimeError(f"core {k} failed") from e

    y = np.empty((N, C_OUT, H, W, KPAIRS), dtype=np.float32)
    for k in range(N_CORES):
        y[:, k * CPC : (k + 1) * CPC] = outs[k]
    return y


# revision 19
# speedup vs baseline: 1.0372x; 1.0372x over previous
"""Trainium2 Bass kernel for nn_ConvLogicLayer.

Computes y[n,c,oy,ox,p] = k0 + ka*A + kb*B + kab*A*B where A/B are
shifted-window gathers of input channels (per the packed `selection`),
and k* are per-(c,p) coefficients derived from softmax(weights) @ OP_COEFFS.

Strategy:
  - Shard C_out (512) across 8 cores -> 64 output channels per core.
  - Each core gets a specialized program: the gather indices and the
    coefficients are baked into the instruction stream (static access
    patterns + immediate scalars), so the kernel is pure streaming
    elementwise work with zero gather traffic.
  - SBUF layout: partition q = n*4 + oyblk (32 images x 4 row-blocks),
    free dim = all 64 input channels x 10 halo rows x 34 padded cols.
    A shifted 8x32 window for any (ch,ry,rx) is then a single static
    3D access pattern on one SBUF tile.
  - Per (c,p) pair: u = kab*B + ka (ScalarE), v = kb*B + k0 (VectorE or
    ScalarE, load-balanced), w = u*A (VectorE), y = w + v written
    p-interleaved (VectorE).  Output DMAd per channel in 512KB chunks
    that are 4KB-contiguous in HBM.
"""

import os
import sys
import threading

import numpy as np

for _p in ("/opt/trn_rl_repo",):
    if _p not in sys.path and os.path.isdir(_p):
        sys.path.insert(0, _p)

import concourse.bass as bass
import concourse.bacc as bacc
import concourse.mybir as mybir
from concourse.tile import TileContext
from concourse import bass_utils

# Problem constants (hardcoded per spec)
N, C_IN, H, W = 32, 64, 32, 32
C_OUT, KPAIRS = 512, 4
N_CORES = 8
CPC = C_OUT // N_CORES  # channels per core

P = 128          # partitions = (n=32) x (oyblk=4)
OYB = 4          # oy blocks per image
OYS = 8          # oy rows per block
HALO = 10        # rows stored per block (8 + 2 halo)
W34 = 34         # padded width
CHSZ = HALO * W34           # 340 elems per (q, channel)
XFREE = C_IN * CHSZ         # 21760 elems per partition
OUT_CSTRIDE = H * W * KPAIRS          # 4096
OUT_NSTRIDE = CPC * OUT_CSTRIDE       # 262144

OP_COEFFS = np.array([
    [0.0, 0.0, 0.0, 0.0], [0.0, 0.0, 0.0, 1.0], [0.0, 1.0, 0.0, -1.0],
    [0.0, 1.0, 0.0, 0.0], [0.0, 0.0, 1.0, -1.0], [0.0, 0.0, 1.0, 0.0],
    [0.0, 1.0, 1.0, -2.0], [0.0, 1.0, 1.0, -1.0], [1.0, -1.0, -1.0, 1.0],
    [1.0, -1.0, -1.0, 2.0], [1.0, 0.0, -1.0, 0.0], [1.0, 0.0, -1.0, 1.0],
    [1.0, -1.0, 0.0, 0.0], [1.0, -1.0, 0.0, 1.0], [1.0, 0.0, 0.0, -1.0],
    [1.0, 0.0, 0.0, 0.0],
], dtype=np.float64)

MULT = mybir.AluOpType.mult
ADD = mybir.AluOpType.add
COPY = mybir.ActivationFunctionType.Copy

# Cost-model ns for load balancing (f32, [128, 256] tiles)
DVE_TT = 327.0   # tensor_tensor, 1x
DVE_TS = 194.0   # tensor_scalar, 2x_2P
ACT_TS = 507.0   # activation, 1x + 352cyc overhead
GPS_TS = 600.0   # gpsimd tensor_scalar (sw impl efficiency ~0.6)
GPS_TT = 600.0   # gpsimd tensor_tensor (sw impl efficiency ~0.42)

# Tuning knobs (A/B'd via TimelineSim; best found = 180.7us/core)
CFG = {
    "use_gps": True,     # offload v/y ops to GPSIMD
    "tp_bufs": 6,
    "yc_bufs": 3,
    "u_act_only": True,  # u always on ScalarE
    "w_dve_only": True,  # w always on VectorE
    "load_chunk_ch": 32,  # split input load for load/compute overlap
}

last_results = [None] * N_CORES  # BassKernelResults per core (for profiling)


def build_core_program(core, ch, ry, rx, coef):
    """One specialized Bass program for `core` (channels core*CPC..+CPC)."""
    nc = bacc.Bacc("TRN2", target_bir_lowering=False)
    xh_d = nc.dram_tensor("xh", [P, XFREE], mybir.dt.float32, kind="ExternalInput")
    out_d = nc.dram_tensor(
        "out", [N, CPC, H, W, KPAIRS], mybir.dt.float32, kind="ExternalOutput"
    )

    use_gps = CFG["use_gps"]
    eng_ns = {"dve": 0.0, "act": 0.0, "gps": 0.0}

    with TileContext(nc) as tc:
        with (
            tc.tile_pool(name="xp", bufs=1) as xpool,
            tc.tile_pool(name="tp", bufs=CFG["tp_bufs"]) as tpool,
            tc.tile_pool(name="yp", bufs=CFG["yc_bufs"]) as ypool,
        ):
            xh = xpool.tile([P, XFREE], mybir.dt.float32)
            # Load input channels in chunks, ordered by first use in the
            # processing sequence, so compute overlaps the streaming load
            # (Tile's subtile tracking scopes each pair's waits to the chunks
            # it actually reads; Bacc splits any multi-wait instructions).
            chunk_ch = CFG.get("load_chunk_ch", 0)
            if chunk_ch <= 0:
                nc.sync.dma_start(xh[:], xh_d[:])
            else:
                first_use = {}
                for cl in range(CPC):
                    c = core * CPC + cl
                    for k in range(2 * KPAIRS):
                        first_use.setdefault(int(ch[c, k]), len(first_use))
                order = sorted(range(C_IN), key=lambda i: first_use.get(i, 1 << 30))
                for gi in range(0, C_IN, chunk_ch):
                    grp = sorted(order[gi : gi + chunk_ch])
                    # merge runs of consecutive channels into single DMAs
                    run = [grp[0]]
                    for cch in grp[1:] + [None]:
                        if cch is not None and cch == run[-1] + 1:
                            run.append(cch)
                            continue
                        lo, hi = run[0] * CHSZ, (run[-1] + 1) * CHSZ
                        nc.sync.dma_start(xh[:, lo:hi], xh_d[:, lo:hi])
                        if cch is not None:
                            run = [cch]
            base = xh[:]
            pitch = base.ap[0][0]
            tens = base.tensor
            base_off = base.offset

            for cl in range(CPC):
                c = core * CPC + cl
                yc = ypool.tile([P, OYS * W * KPAIRS], mybir.dt.float32, tag="yc")
                ybase = yc[:]
                ypitch = ybase.ap[0][0]
                for p4 in range(KPAIRS):
                    ka_, kb_ = 2 * p4, 2 * p4 + 1
                    offA = base_off + int(ch[c, ka_]) * CHSZ + int(ry[c, ka_]) * W34 + int(rx[c, ka_])
                    offB = base_off + int(ch[c, kb_]) * CHSZ + int(ry[c, kb_]) * W34 + int(rx[c, kb_])
                    A_ap = bass.AP(tens, offA, [[pitch, P], [W34, OYS], [1, W]])
                    B_ap = bass.AP(tens, offB, [[pitch, P], [W34, OYS], [1, W]])

                    k0 = float(coef[c, p4, 0])
                    ka = float(coef[c, p4, 1])
                    kb = float(coef[c, p4, 2])
                    kab = float(coef[c, p4, 3])

                    u = tpool.tile([P, OYS * W], mybir.dt.float32, tag="u")
                    v = tpool.tile([P, OYS * W], mybir.dt.float32, tag="v")
                    w = tpool.tile([P, OYS * W], mybir.dt.float32, tag="w")
                    u3 = u[:].rearrange("p (a b) -> p a b", b=W)
                    v3 = v[:].rearrange("p (a b) -> p a b", b=W)
                    w3 = w[:].rearrange("p (a b) -> p a b", b=W)

                    def pick(cands):
                        eng, cost = min(cands, key=lambda c: eng_ns[c[0]] + c[1])
                        eng_ns[eng] += cost
                        return eng

                    # u = kab*B + ka
                    if CFG.get("u_act_only"):
                        ucands = [("act", ACT_TS)]
                    else:
                        ucands = [("act", ACT_TS), ("dve", DVE_TS)]
                        if use_gps:
                            ucands.append(("gps", GPS_TS))
                    ueng = pick(ucands)
                    if ueng == "act":
                        nc.scalar.activation(u3, B_ap, COPY, bias=ka, scale=kab)
                    elif ueng == "gps":
                        nc.gpsimd.tensor_scalar(u3, B_ap, kab, ka, MULT, ADD)
                    else:
                        nc.vector.tensor_scalar(u3, B_ap, kab, ka, MULT, ADD)
                    # v = kb*B + k0
                    vcands = [("dve", DVE_TS), ("act", ACT_TS)]
                    if use_gps:
                        vcands.append(("gps", GPS_TS))
                    veng = pick(vcands)
                    if veng == "act":
                        nc.scalar.activation(v3, B_ap, COPY, bias=k0, scale=kb)
                    elif veng == "gps":
                        nc.gpsimd.tensor_scalar(v3, B_ap, kb, k0, MULT, ADD)
                    else:
                        nc.vector.tensor_scalar(v3, B_ap, kb, k0, MULT, ADD)
                    # w = u * A
                    wcands = [("dve", DVE_TT)]
                    if use_gps and not CFG.get("w_dve_only"):
                        wcands.append(("gps", GPS_TT))
                    weng = pick(wcands)
                    if weng == "gps":
                        nc.gpsimd.tensor_tensor(w3, u3, A_ap, MULT)
                    else:
                        nc.vector.tensor_tensor(w3, u3, A_ap, MULT)
                    # y = w + v, written p-interleaved into yc
                    yap = bass.AP(
                        ybase.tensor, ybase.offset + p4,
                        [[ypitch, P], [W * KPAIRS, OYS], [KPAIRS, W]],
                    )
                    ycands = [("dve", DVE_TT)]
                    if use_gps:
                        ycands.append(("gps", GPS_TT))
                    yeng = pick(ycands)
                    if yeng == "gps":
                        nc.gpsimd.tensor_tensor(yap, w3, v3, ADD)
                    else:
                        nc.vector.tensor_tensor(yap, w3, v3, ADD)

                # DMA this channel out: HBM [n, oyblk, (oy',ox,p)=1024]
                oap = bass.AP(
                    out_d, cl * OUT_CSTRIDE,
                    [[OUT_NSTRIDE, N], [OYS * W * KPAIRS, OYB], [1, OYS * W * KPAIRS]],
                )
                nc.sync.dma_start(oap, yc[:])
    nc.finalize()  # Bacc: splits >1-wait syncs into event semaphores
    return nc


def _prep_inputs(x, weights, selection):
    x = np.ascontiguousarray(np.asarray(x, dtype=np.float32))
    weights = np.asarray(weights, dtype=np.float32)
    selection = np.asarray(selection, dtype=np.int32)

    # coefficients: softmax over 16 logic ops folded into {1,a,b,ab} basis
    w64 = weights.astype(np.float64)
    e = np.exp(w64 - w64.max(axis=-1, keepdims=True))
    prob = e / e.sum(axis=-1, keepdims=True)
    coef = (prob @ OP_COEFFS).astype(np.float32)  # [C_OUT, 4, 4]

    ch = ((selection >> 16) & 0xFFFF).astype(np.int64)
    ry = ((selection >> 8) & 0xFF).astype(np.int64)
    rx = (selection & 0xFF).astype(np.int64)

    # halo layout: xh[q=(n,oyblk), ch, r, w] = xpad[n, ch, oyblk*8+r, w]
    xpad = np.zeros((N, C_IN, H + 2, W + 2), dtype=np.float32)
    xpad[:, :, 1 : H + 1, 1 : W + 1] = x
    xh = np.empty((N, OYB, C_IN, HALO, W34), dtype=np.float32)
    for b in range(OYB):
        xh[:, b] = xpad[:, :, b * OYS : b * OYS + HALO, :]
    xh = np.ascontiguousarray(xh.reshape(P, XFREE))
    return xh, ch, ry, rx, coef


def kernel(x, weights, selection):
    assert x.shape == (N, C_IN, H, W), x.shape
    assert weights.shape == (C_OUT, 4, 16), weights.shape
    assert selection.shape == (C_OUT, 8), selection.shape

    xh, ch, ry, rx, coef = _prep_inputs(x, weights, selection)

    progs = [build_core_program(k, ch, ry, rx, coef) for k in range(N_CORES)]

    import jax

    devices = jax.devices()
    assert len(devices) >= N_CORES, devices

    outs = [None] * N_CORES
    errs = [None] * N_CORES
    # NTFF tracing needs axon hooks that aren't present in this container —
    # make sure run_bass_kernel_spmd never tries (BASS_TRACE in env would).
    os.environ["BASS_NEVER_TRACE"] = "1"

    def run_one(k):
        try:
            with jax.default_device(devices[k]):
                res = bass_utils.run_bass_kernel_spmd(
                    progs[k], [{"xh": xh}], core_ids=[k]
                )
            last_results[k] = res
            outs[k] = res.results[0]["out"]
        except Exception as e:  # noqa: BLE001
            errs[k] = e

    threads = [threading.Thread(target=run_one, args=(k,)) for k in range(N_CORES)]
    for t in threads:
        t.start()
    for t in threads:
        t.join()
    for k, e in enumerate(errs):
        if e is not None:
            raise RuntimeError(f"core {k} failed") from e

    y = np.empty((N, C_OUT, H, W, KPAIRS), dtype=np.float32)
    for k in range(N_CORES):
        y[:, k * CPC : (k + 1) * CPC] = outs[k]
    return y
